# revision 34
# baseline (speedup 1.0000x reference)
"""Trainium2 Bass kernel for nn_Attention_59287728554369.

Multi-head cross-attention, b=2, nq=nk=2048, 16 heads x 64 dim, d_model=1024.
Sharding: batch (2) x head-groups (4 heads each) -> 8 cores.
Each core computes q/k/v projections for its 4 heads, fused masked softmax
attention, and a partial output projection; host sums the 4 partials per batch.

Key optimizations:
- fp16 activations/weights/IO (f32 PSUM accumulation): halves HBM traffic
- masked keys are compacted away on the host (exact: they contribute
  exp(-inf)=0 anyway); the kernel is shape-specialized to the compacted
  key count rounded up to 128 (padding keys get bias -1e9 -> exp == 0)
- batched DMAs: one descriptor-chain per 512-column activation block
  (8 k-tiles in one instruction) and per 128-row output block
- input DMAs on the SP HWDGE ring in consumption order; vones/wo and all
  output stores on the ACT ring (no head-of-line blocking)
- softmax exp fused on ACT with per-partition mask bias, exp -> fp16
- denominators via a ones-augmented V column in the same PV matmul
- normalization: DVE copy + gpsimd partition-broadcast + fast reciprocal
- Q-projection and out-projection halves interleaved at every attention
  block boundary so PE always has independent work while ACT drains the
  exp tail
"""
import os
import sys

sys.path.insert(0, "/opt/trn_rl_repo")

import numpy as np

import concourse.bass as bass  # noqa: F401
import concourse.tile as tile
from concourse import bacc, mybir

F32 = mybir.dt.float32
F16 = mybir.dt.float16
AF = mybir.ActivationFunctionType

# Problem constants (hardcoded per contest rules)
B = 2
NQ = 2048
NK = 2048
D = 1024          # d_model
H = 16            # total heads
DH = 64           # head dim
HG = 4            # heads per core
CG = HG * DH      # channels per core = 256
N_CORES = 8
SCALE = DH ** -0.5

_CACHE = {}


def build_nc(reps=1, nkc=NK, py_unroll=False):
    """Build the single-core Bass program (identical across cores).

    nkc: compacted key count (multiple of 128, <= NK).
    reps>1 wraps the computation in an on-device For_i loop (same buffers) so
    test harnesses can measure marginal wall time per rep = HW exec time.
    py_unroll=True unrolls reps in Python instead (for simulator use).
    """
    assert nkc % 128 == 0 and 128 <= nkc <= NK
    JTC = nkc // 128               # 128-wide j tiles
    # j blocks for the projections: full 512s plus one remainder block
    jblocks = [(s, 512) for s in range(0, nkc - nkc % 512, 512)]
    if nkc % 512:
        jblocks.append((nkc - nkc % 512, nkc % 512))

    nc = bacc.Bacc("TRN2", target_bir_lowering=False, debug=False)

    qT = nc.dram_tensor("qT", [D, NQ], F16, kind="ExternalInput").ap()
    cT = nc.dram_tensor("cT", [D, nkc], F16, kind="ExternalInput").ap()
    wq = nc.dram_tensor("wq", [D, CG], F16, kind="ExternalInput").ap()
    wk = nc.dram_tensor("wk", [D, CG], F16, kind="ExternalInput").ap()
    wv = nc.dram_tensor("wv", [D, CG], F16, kind="ExternalInput").ap()
    wo = nc.dram_tensor("wo", [CG, D], F16, kind="ExternalInput").ap()
    kb = nc.dram_tensor("kb", [128, JTC], F32, kind="ExternalInput").ap()
    vones = nc.dram_tensor("vones", [128, JTC * HG], F16, kind="ExternalInput").ap()
    outp = nc.dram_tensor("outp", [NQ, D], F16, kind="ExternalOutput").ap()

    qTr = qT.rearrange("(t p) i -> p t i", p=128)
    cTr = cT.rearrange("(t p) j -> p t j", p=128)

    KT = 8   # k tiles over d_model
    IB = 4   # 512-wide i blocks

    with tile.TileContext(nc) as tc:
        with tc.tile_pool(name="sb", bufs=1) as sb:
            # ---- persistent SBUF tensors; issue order = consumption order.
            # vones/wo go on the ACT HWDGE ring so they never delay the
            # SP-ring input stream.
            kb_sb = sb.tile([128, JTC], F32, bufs=1)
            wq_sb = sb.tile([128, KT, CG], F16, bufs=1)
            nc.sync.dma_start(out=wq_sb, in_=wq.rearrange("(t p) c -> p t c", p=128))

            # projected K^T / Q^T: head pair per tile
            kt_sb = [sb.tile([128, nkc], F16, bufs=1, name=f"kt{i}") for i in range(2)]
            qt_sb = [sb.tile([128, NQ], F16, bufs=1, name=f"qt{i}") for i in range(2)]
            # V (+ones col): [j, head-major 4x65]
            v_sb = sb.tile([128, JTC, HG * 65], F16, bufs=1)
            wk_sb = sb.tile([128, KT, CG], F16, bufs=1)
            wv_sb = sb.tile([128, KT, CG], F16, bufs=1)
            wo_sb = sb.tile([128, 2, D], F16, bufs=1)
            # normalized attention output O^T per head pair: [128, 2048]
            ot_sb = [sb.tile([128, NQ], F16, bufs=1, name=f"ot{i}") for i in range(2)]

            def _one_pass():
                with tc.tile_pool(name="ps", bufs=1, space="PSUM") as ps:
                    def _qt_dma(ib2):
                        a = sb.tile([128, KT, 512], F16, tag="act", bufs=6, name="act")
                        # two halves so the first matmuls start at half the
                        # transfer latency
                        for h in range(2):
                            nc.sync.dma_start(
                                out=a[:, h * 4:(h + 1) * 4],
                                in_=qTr[:, h * 4:(h + 1) * 4,
                                        ib2 * 512:(ib2 + 1) * 512],
                            )
                        return a

                    def _qt_mm(ib2, qx, cb):
                        qt_ps = ps.tile([128, 512], F32, tag="kt", bufs=2, name="qt_ps")
                        for k in range(KT):
                            nc.tensor.matmul(
                                qt_ps,
                                wq_sb[:, k, cb * 128:(cb + 1) * 128],
                                qx[:, k],
                                start=(k == 0),
                                stop=(k == KT - 1),
                            )
                        nc.vector.tensor_copy(
                            qt_sb[cb][:, ib2 * 512:(ib2 + 1) * 512], qt_ps
                        )

                    def _alloc_pvs():
                        out = []
                        for b in range(2):
                            pv = ps.tile([65, 512], F32, tag="pv", bufs=2, name="pv")
                            out.append(pv)
                        return out

                    def _emit_st(hp, ib2, jt):
                        st = ps.tile([128, 1024], F32, tag="st", bufs=2, name="st")
                        for b in range(2):
                            nc.tensor.matmul(
                                st[:, b * 512:(b + 1) * 512],
                                kt_sb[hp][b * 64:(b + 1) * 64, jt * 128:(jt + 1) * 128],
                                qt_sb[hp][b * 64:(b + 1) * 64, ib2 * 512:(ib2 + 1) * 512],
                                start=True,
                                stop=True,
                            )
                        return st

                    def _exp(jt, st):
                        e = sb.tile([128, 1024], F16, tag="et", bufs=4, name="e")
                        nc.scalar.activation(
                            e, st, AF.Exp, bias=kb_sb[:, jt:jt + 1], scale=SCALE
                        )
                        return e

                    def _pv_step(hp, jt, e, pvs):
                        for b in range(2):
                            h = 2 * hp + b
                            nc.tensor.matmul(
                                pvs[b],
                                v_sb[:, jt, h * 65:(h + 1) * 65],
                                e[:, b * 512:(b + 1) * 512],
                                start=(jt == 0),
                                stop=(jt == JTC - 1),
                            )

                    def _normalize(hp, ib2, pvs, tail=False):
                        pvcs, recs = [], []
                        for b in range(2):
                            dr = sb.tile([1, 512], F32, tag="dr", bufs=2, name="dr")
                            nc.vector.tensor_copy(dr, pvs[b][64:65, :])
                            pvc = sb.tile([64, 512], F32, tag="pvc", bufs=4, name="pvc")
                            nc.vector.tensor_copy(pvc, pvs[b][0:64, :])
                            den = sb.tile([64, 512], F32, tag="den", bufs=2, name="den")
                            nc.gpsimd.partition_broadcast(den, dr[0:1, :])
                            rec = sb.tile([64, 512], F32, tag="rec", bufs=2, name="rec")
                            nc.vector.reciprocal_approx_fast(out=rec, in_=den)
                            pvcs.append(pvc)
                            recs.append(rec)
                            if not tail:
                                nc.vector.tensor_mul(
                                    ot_sb[hp][b * 64:(b + 1) * 64,
                                              ib2 * 512:(ib2 + 1) * 512],
                                    pvc,
                                    rec,
                                )
                        if tail:
                            # 128-col chunks, both head-halves per chunk, so
                            # the final out-projection starts on chunk 0 while
                            # later chunks are still normalizing
                            for c in range(4):
                                cs = slice(c * 128, (c + 1) * 128)
                                for b in range(2):
                                    nc.vector.tensor_mul(
                                        ot_sb[hp][b * 64:(b + 1) * 64,
                                                  ib2 * 512 + c * 128:
                                                  ib2 * 512 + (c + 1) * 128],
                                        pvcs[b][:, cs],
                                        recs[b][:, cs],
                                    )

                    def _exp_half(jt, st, half):
                        e = sb.tile([128, 512], F16, tag="eth", bufs=2, name="eh")
                        nc.scalar.activation(
                            e, st[:, half * 512:(half + 1) * 512], AF.Exp,
                            bias=kb_sb[:, jt:jt + 1], scale=SCALE
                        )
                        return e

                    def _attn_block(hp, ib2, sts, next_blk):
                        pvs = _alloc_pvs()
                        nxt = {}
                        for jt in range(JTC):
                            e = _exp(jt, sts.pop(jt))
                            if jt + 2 < JTC:
                                sts[jt + 2] = _emit_st(hp, ib2, jt + 2)
                            _pv_step(hp, jt, e, pvs)
                        _normalize(hp, ib2, pvs, tail=(next_blk is None))
                        if next_blk is not None:
                            for j2 in range(2):
                                nxt[j2] = _emit_st(next_blk[0], next_blk[1], j2)
                        return nxt

                    def _oproj_half(ib2, half, tail=False):
                        # PSUM->SBUF copies on the (otherwise idle) Pool
                        # engine: DVE stays free for the normalize chain.
                        # At the pass tail, copies alternate Pool/DVE and
                        # the store is split per half so the last DMA
                        # starts as early as possible.
                        for it in (0, 1):
                            itg = ib2 * 4 + half * 2 + it
                            osb = sb.tile([128, 1024], F16, tag="osb", bufs=3, name="osb")
                            for m in range(2):
                                op = ps.tile([128, 512], F32, tag="kt", bufs=2, name="op")
                                for kk in range(2):
                                    nc.tensor.matmul(
                                        op,
                                        ot_sb[kk][:, itg * 128:(itg + 1) * 128],
                                        wo_sb[:, kk, m * 512:(m + 1) * 512],
                                        start=(kk == 0),
                                        stop=(kk == 1),
                                    )
                                eng = nc.vector
                                eng.tensor_copy(osb[:, m * 512:(m + 1) * 512], op)
                                if tail:
                                    nc.scalar.dma_start(
                                        out=outp[itg * 128:(itg + 1) * 128,
                                                 m * 512:(m + 1) * 512],
                                        in_=osb[:, m * 512:(m + 1) * 512],
                                    )
                            if not tail:
                                nc.scalar.dma_start(
                                    out=outp[itg * 128:(itg + 1) * 128, :], in_=osb
                                )

                    # ---- phase 1a: Q^T block 0, then K^T and V from the
                    # compacted context (input DMAs interleaved on the SP
                    # ring in exactly the order the PE consumes them) ----
                    qx = {0: _qt_dma(0)}
                    _qt_mm(0, qx[0], 0)
                    _qt_mm(0, qx[0], 1)
                    ct_tiles = []
                    nc.sync.dma_start(
                        out=wk_sb, in_=wk.rearrange("(t p) c -> p t c", p=128)
                    )
                    for bi, (j0, bw) in enumerate(jblocks):
                        ct = sb.tile([128, KT, 512], F16, tag="act", bufs=6, name="act")
                        for h in range(2):
                            nc.sync.dma_start(
                                out=ct[:, h * 4:(h + 1) * 4, 0:bw],
                                in_=cTr[:, h * 4:(h + 1) * 4, j0:j0 + bw],
                            )
                        ct_tiles.append(ct)
                        if bi == 0:
                            nc.sync.dma_start(
                                out=wv_sb, in_=wv.rearrange("(t p) c -> p t c", p=128)
                            )
                    # late-consumed tensors go on the SP ring AFTER the
                    # startup-critical inputs: issue order = transfer order,
                    # so they never delay wq/qx0/wk/ct
                    nc.sync.dma_start(out=kb_sb, in_=kb)
                    nc.sync.dma_start(
                        out=v_sb.rearrange("p t (h e) -> p t h e", e=65)[:, :, :, 64:65],
                        in_=vones.rearrange("p (t h) -> p t h", h=HG)[:, :, :, None],
                    )
                    nc.sync.dma_start(
                        out=wo_sb, in_=wo.rearrange("(t p) m -> p t m", p=128)
                    )
                    qx[1] = _qt_dma(1)
                    qx[2] = _qt_dma(2)
                    qx[3] = _qt_dma(3)
                    for (j0, bw), ct in zip(jblocks, ct_tiles):
                        for cb in range(2):
                            kt_ps = ps.tile([128, 512], F32, tag="kt", bufs=2, name="kt_ps")
                            for k in range(KT):
                                nc.tensor.matmul(
                                    kt_ps[:, 0:bw],
                                    wk_sb[:, k, cb * 128:(cb + 1) * 128],
                                    ct[:, k, 0:bw],
                                    start=(k == 0),
                                    stop=(k == KT - 1),
                                )
                            nc.vector.tensor_copy(
                                kt_sb[cb][:, j0:j0 + bw], kt_ps[:, 0:bw]
                            )
                        for js in range(bw // 128):
                            v_ps = ps.tile([128, CG], F32, tag="pv", bufs=2, name="v_ps")
                            for k in range(KT):
                                nc.tensor.matmul(
                                    v_ps,
                                    ct[:, k, js * 128:(js + 1) * 128],
                                    wv_sb[:, k, :],
                                    start=(k == 0),
                                    stop=(k == KT - 1),
                                )
                            nc.vector.tensor_copy(
                                v_sb[:, j0 // 128 + js].rearrange(
                                    "p (h e) -> p h e", e=65
                                )[:, :, 0:64],
                                v_ps.rearrange("p (h e) -> p h e", e=64),
                            )

                    # ---- per i block: attention with ~2-3.4us of
                    # independent PE work (Q^T projection / out-projection
                    # halves) inserted at every head-pair boundary so PE
                    # never waits for ACT to drain the exp tail.  Fill work
                    # only ever uses outputs at least one full i-block old.
                    blocks = [(hp, ib2) for ib2 in range(IB) for hp in (0, 1)]

                    def _fill(bnd):
                        if bnd == 1:
                            _qt_mm(1, qx[1], 0)
                            _qt_mm(1, qx[1], 1)
                        elif bnd == 2:
                            _qt_mm(2, qx[2], 0)
                            _qt_mm(2, qx[2], 1)
                        elif bnd == 3:
                            _oproj_half(0, 0)
                            _oproj_half(0, 1)
                        elif bnd == 4:
                            _qt_mm(3, qx[3], 0)
                        elif bnd == 6:
                            _qt_mm(3, qx[3], 1)
                        elif bnd == 5:
                            _oproj_half(1, 0)
                            _oproj_half(1, 1)
                        elif bnd == 7:
                            _oproj_half(2, 0)
                        elif bnd == 8:
                            _oproj_half(2, 1)

                    sts = {j: _emit_st(0, 0, j) for j in range(2)}
                    for bi, blk in enumerate(blocks):
                        nxt = blocks[bi + 1] if bi + 1 < len(blocks) else None
                        nsts = _attn_block(blk[0], blk[1], sts, nxt)
                        _fill(bi + 1)
                        sts = nsts
                    _oproj_half(3, 0, tail=True)
                    _oproj_half(3, 1, tail=True)

            if reps == 1:
                _one_pass()
            elif py_unroll:
                for _ in range(reps):
                    _one_pass()
            else:
                with tc.For_i(0, reps, 1):
                    _one_pass()

    nc.compile()
    return nc


def _nkc_for_mask(mask):
    """Compacted key count: max unmasked keys over batches, rounded to 128."""
    counts = [int((~mask[bi]).sum()) for bi in range(mask.shape[0])]
    nkc = max(max(counts), 1)
    nkc = min(((nkc + 127) // 128) * 128, NK)
    return nkc


def _prep_core_inputs(q, context, mask, Wq, Wkv, Wout, core, nkc=NK):
    bi, g = core // 4, core % 4
    c0 = g * CG
    JTC = nkc // 128
    keep_idx = np.nonzero(~mask[bi])[0]
    ctx_c = np.zeros((nkc, D), dtype=np.float16)
    ctx_c[: len(keep_idx)] = context[bi][keep_idx]
    kbias = np.full(nkc, np.float32(-1e9), dtype=np.float32)
    kbias[: len(keep_idx)] = 0.0
    return {
        "qT": np.ascontiguousarray(q[bi].T.astype(np.float16)),
        "cT": np.ascontiguousarray(ctx_c.T),
        "wq": np.ascontiguousarray(Wq[:, c0:c0 + CG].astype(np.float16)),
        "wk": np.ascontiguousarray(Wkv[:, c0:c0 + CG].astype(np.float16)),
        "wv": np.ascontiguousarray(Wkv[:, D + c0:D + c0 + CG].astype(np.float16)),
        "wo": np.ascontiguousarray(Wout[c0:c0 + CG, :].astype(np.float16)),
        "kb": np.ascontiguousarray(kbias.reshape(JTC, 128).T),
        "vones": np.ones((128, JTC * HG), dtype=np.float16),
    }


def kernel(q, context, mask, Wq, Wkv, Wout, b_out):
    from concourse.bass_utils import run_bass_kernel_spmd

    q = np.asarray(q, dtype=np.float32)
    context = np.asarray(context, dtype=np.float32)
    mask = np.asarray(mask)
    Wq = np.asarray(Wq, dtype=np.float32)
    Wkv = np.asarray(Wkv, dtype=np.float32)
    Wout = np.asarray(Wout, dtype=np.float32)
    b_out = np.asarray(b_out, dtype=np.float32)

    nkc = _nkc_for_mask(mask)
    key = ("nc", nkc)
    if key not in _CACHE:
        _CACHE[key] = build_nc(nkc=nkc)
    nc = _CACHE[key]
    _CACHE["nc"] = nc
    _CACHE["nkc"] = nkc

    in_maps = [
        _prep_core_inputs(q, context, mask, Wq, Wkv, Wout, c, nkc=nkc)
        for c in range(N_CORES)
    ]

    trace = bool(int(os.environ.get("BASS_ATTN_TRACE", "0")))
    res = run_bass_kernel_spmd(nc, in_maps, list(range(N_CORES)), trace=trace)
    _CACHE["last_results"] = res
    _CACHE["last_in_maps"] = in_maps

    out = np.empty((B, NQ, D), dtype=np.float32)
    for bi in range(B):
        acc = res.results[4 * bi]["outp"].astype(np.float32).copy()
        for g in range(1, 4):
            acc += res.results[4 * bi + g]["outp"].astype(np.float32)
        out[bi] = acc + b_out[None, :]
    return out


# revision 35
# speedup vs baseline: 1.0047x; 1.0047x over previous
"""Trainium2 Bass kernel for nn_Attention_59287728554369.

Multi-head cross-attention, b=2, nq=nk=2048, 16 heads x 64 dim, d_model=1024.
Sharding: batch (2) x head-groups (4 heads each) -> 8 cores.
Each core computes q/k/v projections for its 4 heads, fused masked softmax
attention, and a partial output projection; host sums the 4 partials per batch.

Key optimizations:
- fp16 activations/weights/IO (f32 PSUM accumulation): halves HBM traffic
- masked keys are compacted away on the host (exact: they contribute
  exp(-inf)=0 anyway); the kernel is shape-specialized to the compacted
  key count rounded up to 128 (padding keys get bias -1e9 -> exp == 0)
- batched DMAs: one descriptor-chain per 512-column activation block
  (8 k-tiles in one instruction) and per 128-row output block
- input DMAs on the SP HWDGE ring in consumption order; vones/wo and all
  output stores on the ACT ring (no head-of-line blocking)
- softmax exp fused on ACT with per-partition mask bias, exp -> fp16
- denominators via a ones-augmented V column in the same PV matmul
- normalization: DVE copy + gpsimd partition-broadcast + fast reciprocal
- Q-projection and out-projection halves interleaved at every attention
  block boundary so PE always has independent work while ACT drains the
  exp tail
"""
import os
import sys

sys.path.insert(0, "/opt/trn_rl_repo")

import numpy as np

import concourse.bass as bass  # noqa: F401
import concourse.tile as tile
from concourse import bacc, mybir

F32 = mybir.dt.float32
F16 = mybir.dt.float16
AF = mybir.ActivationFunctionType

# Problem constants (hardcoded per contest rules)
B = 2
NQ = 2048
NK = 2048
D = 1024          # d_model
H = 16            # total heads
DH = 64           # head dim
HG = 4            # heads per core
CG = HG * DH      # channels per core = 256
N_CORES = 8
SCALE = DH ** -0.5

_CACHE = {}


def build_nc(reps=1, nkc=NK, py_unroll=False):
    """Build the single-core Bass program (identical across cores).

    nkc: compacted key count (multiple of 128, <= NK).
    reps>1 wraps the computation in an on-device For_i loop (same buffers) so
    test harnesses can measure marginal wall time per rep = HW exec time.
    py_unroll=True unrolls reps in Python instead (for simulator use).
    """
    assert nkc % 128 == 0 and 128 <= nkc <= NK
    JTC = nkc // 128               # 128-wide j tiles
    # j blocks for the projections: full 512s plus one remainder block
    jblocks = [(s, 512) for s in range(0, nkc - nkc % 512, 512)]
    if nkc % 512:
        jblocks.append((nkc - nkc % 512, nkc % 512))

    nc = bacc.Bacc("TRN2", target_bir_lowering=False, debug=False)

    qT = nc.dram_tensor("qT", [D, NQ], F16, kind="ExternalInput").ap()
    cT = nc.dram_tensor("cT", [D, nkc], F16, kind="ExternalInput").ap()
    wq = nc.dram_tensor("wq", [D, CG], F16, kind="ExternalInput").ap()
    wk = nc.dram_tensor("wk", [D, CG], F16, kind="ExternalInput").ap()
    wv = nc.dram_tensor("wv", [D, CG], F16, kind="ExternalInput").ap()
    wo = nc.dram_tensor("wo", [CG, D], F16, kind="ExternalInput").ap()
    kb = nc.dram_tensor("kb", [128, JTC], F32, kind="ExternalInput").ap()
    outp = nc.dram_tensor("outp", [NQ, D], F16, kind="ExternalOutput").ap()

    qTr = qT.rearrange("(t p) i -> p t i", p=128)
    cTr = cT.rearrange("(t p) j -> p t j", p=128)

    KT = 8   # k tiles over d_model
    IB = 4   # 512-wide i blocks

    with tile.TileContext(nc) as tc:
        with tc.tile_pool(name="sb", bufs=1) as sb:
            # ---- persistent SBUF tensors; issue order = consumption order.
            # vones/wo go on the ACT HWDGE ring so they never delay the
            # SP-ring input stream.
            kb_sb = sb.tile([128, JTC], F32, bufs=1)
            wq_sb = sb.tile([128, KT, CG], F16, bufs=1)
            nc.sync.dma_start(out=wq_sb, in_=wq.rearrange("(t p) c -> p t c", p=128))

            # projected K^T / Q^T: head pair per tile
            kt_sb = [sb.tile([128, nkc], F16, bufs=1, name=f"kt{i}") for i in range(2)]
            qt_sb = [sb.tile([128, NQ], F16, bufs=1, name=f"qt{i}") for i in range(2)]
            # V (+ones col): [j, head-major 4x65]
            v_sb = sb.tile([128, JTC, HG * 65], F16, bufs=1)
            wk_sb = sb.tile([128, KT, CG], F16, bufs=1)
            wv_sb = sb.tile([128, KT, CG], F16, bufs=1)
            wo_sb = sb.tile([128, 2, D], F16, bufs=1)
            # ones column of V written on-device once (a DMA here is a
            # 6656-descriptor scatter that loses the race against the first
            # PV matmul as the pass gets faster)
            nc.gpsimd.memset(
                v_sb.rearrange("p t (h e) -> p t h e", e=65)[:, :, :, 64:65], 1.0
            )
            # normalized attention output O^T per head pair: [128, 2048]
            ot_sb = [sb.tile([128, NQ], F16, bufs=1, name=f"ot{i}") for i in range(2)]

            def _one_pass():
                with tc.tile_pool(name="ps", bufs=1, space="PSUM") as ps:
                    def _qt_dma(ib2):
                        a = sb.tile([128, KT, 512], F16, tag="act", bufs=6, name="act")
                        # two halves so the first matmuls start at half the
                        # transfer latency
                        for h in range(2):
                            nc.sync.dma_start(
                                out=a[:, h * 4:(h + 1) * 4],
                                in_=qTr[:, h * 4:(h + 1) * 4,
                                        ib2 * 512:(ib2 + 1) * 512],
                            )
                        return a

                    def _qt_mm(ib2, qx, cb):
                        qt_ps = ps.tile([128, 512], F32, tag="kt", bufs=2, name="qt_ps")
                        for k in range(KT):
                            nc.tensor.matmul(
                                qt_ps,
                                wq_sb[:, k, cb * 128:(cb + 1) * 128],
                                qx[:, k],
                                start=(k == 0),
                                stop=(k == KT - 1),
                            )
                        nc.vector.tensor_copy(
                            qt_sb[cb][:, ib2 * 512:(ib2 + 1) * 512], qt_ps
                        )

                    def _alloc_pvs():
                        out = []
                        for b in range(2):
                            pv = ps.tile([65, 512], F32, tag="pv", bufs=2, name="pv")
                            out.append(pv)
                        return out

                    def _emit_st(hp, ib2, jt):
                        st = ps.tile([128, 1024], F32, tag="st", bufs=2, name="st")
                        for b in range(2):
                            nc.tensor.matmul(
                                st[:, b * 512:(b + 1) * 512],
                                kt_sb[hp][b * 64:(b + 1) * 64, jt * 128:(jt + 1) * 128],
                                qt_sb[hp][b * 64:(b + 1) * 64, ib2 * 512:(ib2 + 1) * 512],
                                start=True,
                                stop=True,
                            )
                        return st

                    def _exp(jt, st):
                        e = sb.tile([128, 1024], F16, tag="et", bufs=4, name="e")
                        nc.scalar.activation(
                            e, st, AF.Exp, bias=kb_sb[:, jt:jt + 1], scale=SCALE
                        )
                        return e

                    def _pv_step(hp, jt, e, pvs):
                        for b in range(2):
                            h = 2 * hp + b
                            nc.tensor.matmul(
                                pvs[b],
                                v_sb[:, jt, h * 65:(h + 1) * 65],
                                e[:, b * 512:(b + 1) * 512],
                                start=(jt == 0),
                                stop=(jt == JTC - 1),
                            )

                    def _normalize(hp, ib2, pvs, tail=False):
                        pvcs, recs = [], []
                        for b in range(2):
                            dr = sb.tile([1, 512], F32, tag="dr", bufs=2, name="dr")
                            nc.vector.tensor_copy(dr, pvs[b][64:65, :])
                            pvc = sb.tile([64, 512], F32, tag="pvc", bufs=4, name="pvc")
                            nc.vector.tensor_copy(pvc, pvs[b][0:64, :])
                            den = sb.tile([64, 512], F32, tag="den", bufs=2, name="den")
                            nc.gpsimd.partition_broadcast(den, dr[0:1, :])
                            rec = sb.tile([64, 512], F32, tag="rec", bufs=2, name="rec")
                            nc.vector.reciprocal_approx_fast(out=rec, in_=den)
                            pvcs.append(pvc)
                            recs.append(rec)
                            if not tail:
                                nc.vector.tensor_mul(
                                    ot_sb[hp][b * 64:(b + 1) * 64,
                                              ib2 * 512:(ib2 + 1) * 512],
                                    pvc,
                                    rec,
                                )
                        if tail:
                            # 128-col chunks, both head-halves per chunk, so
                            # the final out-projection starts on chunk 0 while
                            # later chunks are still normalizing
                            for c in range(4):
                                cs = slice(c * 128, (c + 1) * 128)
                                for b in range(2):
                                    nc.vector.tensor_mul(
                                        ot_sb[hp][b * 64:(b + 1) * 64,
                                                  ib2 * 512 + c * 128:
                                                  ib2 * 512 + (c + 1) * 128],
                                        pvcs[b][:, cs],
                                        recs[b][:, cs],
                                    )

                    def _exp_half(jt, st, half):
                        e = sb.tile([128, 512], F16, tag="eth", bufs=2, name="eh")
                        nc.scalar.activation(
                            e, st[:, half * 512:(half + 1) * 512], AF.Exp,
                            bias=kb_sb[:, jt:jt + 1], scale=SCALE
                        )
                        return e

                    def _attn_block(hp, ib2, sts, next_blk):
                        pvs = _alloc_pvs()
                        nxt = {}
                        for jt in range(JTC):
                            e = _exp(jt, sts.pop(jt))
                            if jt + 2 < JTC:
                                sts[jt + 2] = _emit_st(hp, ib2, jt + 2)
                            _pv_step(hp, jt, e, pvs)
                        _normalize(hp, ib2, pvs, tail=(next_blk is None))
                        if next_blk is not None:
                            for j2 in range(2):
                                nxt[j2] = _emit_st(next_blk[0], next_blk[1], j2)
                        return nxt

                    def _oproj_half(ib2, half, tail=False):
                        # PSUM->SBUF copies on the (otherwise idle) Pool
                        # engine: DVE stays free for the normalize chain.
                        # At the pass tail, copies alternate Pool/DVE and
                        # the store is split per half so the last DMA
                        # starts as early as possible.
                        for it in (0, 1):
                            itg = ib2 * 4 + half * 2 + it
                            osb = sb.tile([128, 1024], F16, tag="osb", bufs=3, name="osb")
                            for m in range(2):
                                op = ps.tile([128, 512], F32, tag="kt", bufs=2, name="op")
                                for kk in range(2):
                                    nc.tensor.matmul(
                                        op,
                                        ot_sb[kk][:, itg * 128:(itg + 1) * 128],
                                        wo_sb[:, kk, m * 512:(m + 1) * 512],
                                        start=(kk == 0),
                                        stop=(kk == 1),
                                    )
                                eng = nc.vector
                                eng.tensor_copy(osb[:, m * 512:(m + 1) * 512], op)
                                if tail:
                                    nc.scalar.dma_start(
                                        out=outp[itg * 128:(itg + 1) * 128,
                                                 m * 512:(m + 1) * 512],
                                        in_=osb[:, m * 512:(m + 1) * 512],
                                    )
                            if not tail:
                                nc.scalar.dma_start(
                                    out=outp[itg * 128:(itg + 1) * 128, :], in_=osb
                                )

                    # ---- phase 1a: Q^T block 0, then K^T and V from the
                    # compacted context (input DMAs interleaved on the SP
                    # ring in exactly the order the PE consumes them) ----
                    qx = {0: _qt_dma(0)}
                    _qt_mm(0, qx[0], 0)
                    _qt_mm(0, qx[0], 1)
                    ct_tiles = []
                    nc.sync.dma_start(
                        out=wk_sb, in_=wk.rearrange("(t p) c -> p t c", p=128)
                    )
                    for bi, (j0, bw) in enumerate(jblocks):
                        ct = sb.tile([128, KT, 512], F16, tag="act", bufs=6, name="act")
                        for h in range(2):
                            nc.sync.dma_start(
                                out=ct[:, h * 4:(h + 1) * 4, 0:bw],
                                in_=cTr[:, h * 4:(h + 1) * 4, j0:j0 + bw],
                            )
                        ct_tiles.append(ct)
                        if bi == 0:
                            nc.sync.dma_start(
                                out=wv_sb, in_=wv.rearrange("(t p) c -> p t c", p=128)
                            )
                    # late-consumed tensors go on the SP ring AFTER the
                    # startup-critical inputs: issue order = transfer order,
                    # so they never delay wq/qx0/wk/ct
                    nc.sync.dma_start(out=kb_sb, in_=kb)
                    nc.sync.dma_start(
                        out=wo_sb, in_=wo.rearrange("(t p) m -> p t m", p=128)
                    )
                    qx[1] = _qt_dma(1)
                    qx[2] = _qt_dma(2)
                    qx[3] = _qt_dma(3)
                    for (j0, bw), ct in zip(jblocks, ct_tiles):
                        for cb in range(2):
                            kt_ps = ps.tile([128, 512], F32, tag="kt", bufs=2, name="kt_ps")
                            for k in range(KT):
                                nc.tensor.matmul(
                                    kt_ps[:, 0:bw],
                                    wk_sb[:, k, cb * 128:(cb + 1) * 128],
                                    ct[:, k, 0:bw],
                                    start=(k == 0),
                                    stop=(k == KT - 1),
                                )
                            nc.vector.tensor_copy(
                                kt_sb[cb][:, j0:j0 + bw], kt_ps[:, 0:bw]
                            )
                        for js in range(bw // 128):
                            v_ps = ps.tile([128, CG], F32, tag="pv", bufs=2, name="v_ps")
                            for k in range(KT):
                                nc.tensor.matmul(
                                    v_ps,
                                    ct[:, k, js * 128:(js + 1) * 128],
                                    wv_sb[:, k, :],
                                    start=(k == 0),
                                    stop=(k == KT - 1),
                                )
                            nc.vector.tensor_copy(
                                v_sb[:, j0 // 128 + js].rearrange(
                                    "p (h e) -> p h e", e=65
                                )[:, :, 0:64],
                                v_ps.rearrange("p (h e) -> p h e", e=64),
                            )

                    # ---- per i block: attention with ~2-3.4us of
                    # independent PE work (Q^T projection / out-projection
                    # halves) inserted at every head-pair boundary so PE
                    # never waits for ACT to drain the exp tail.  Fill work
                    # only ever uses outputs at least one full i-block old.
                    blocks = [(hp, ib2) for ib2 in range(IB) for hp in (0, 1)]

                    def _fill(bnd):
                        if bnd == 1:
                            _qt_mm(1, qx[1], 0)
                            _qt_mm(1, qx[1], 1)
                        elif bnd == 2:
                            _qt_mm(2, qx[2], 0)
                            _qt_mm(2, qx[2], 1)
                        elif bnd == 3:
                            _oproj_half(0, 0)
                            _oproj_half(0, 1)
                        elif bnd == 4:
                            _qt_mm(3, qx[3], 0)
                        elif bnd == 6:
                            _qt_mm(3, qx[3], 1)
                        elif bnd == 5:
                            _oproj_half(1, 0)
                            _oproj_half(1, 1)
                        elif bnd == 7:
                            _oproj_half(2, 0)
                        elif bnd == 8:
                            _oproj_half(2, 1)

                    sts = {j: _emit_st(0, 0, j) for j in range(2)}
                    for bi, blk in enumerate(blocks):
                        nxt = blocks[bi + 1] if bi + 1 < len(blocks) else None
                        nsts = _attn_block(blk[0], blk[1], sts, nxt)
                        _fill(bi + 1)
                        sts = nsts
                    _oproj_half(3, 0, tail=True)
                    _oproj_half(3, 1, tail=True)

            if reps == 1:
                _one_pass()
            elif py_unroll:
                for _ in range(reps):
                    _one_pass()
            else:
                with tc.For_i(0, reps, 1):
                    _one_pass()

    nc.compile()
    return nc


def _nkc_for_mask(mask):
    """Compacted key count: max unmasked keys over batches, rounded to 128."""
    counts = [int((~mask[bi]).sum()) for bi in range(mask.shape[0])]
    nkc = max(max(counts), 1)
    nkc = min(((nkc + 127) // 128) * 128, NK)
    return nkc


def _prep_core_inputs(q, context, mask, Wq, Wkv, Wout, core, nkc=NK):
    bi, g = core // 4, core % 4
    c0 = g * CG
    JTC = nkc // 128
    keep_idx = np.nonzero(~mask[bi])[0]
    ctx_c = np.zeros((nkc, D), dtype=np.float16)
    ctx_c[: len(keep_idx)] = context[bi][keep_idx]
    kbias = np.full(nkc, np.float32(-1e9), dtype=np.float32)
    kbias[: len(keep_idx)] = 0.0
    return {
        "qT": np.ascontiguousarray(q[bi].T.astype(np.float16)),
        "cT": np.ascontiguousarray(ctx_c.T),
        "wq": np.ascontiguousarray(Wq[:, c0:c0 + CG].astype(np.float16)),
        "wk": np.ascontiguousarray(Wkv[:, c0:c0 + CG].astype(np.float16)),
        "wv": np.ascontiguousarray(Wkv[:, D + c0:D + c0 + CG].astype(np.float16)),
        "wo": np.ascontiguousarray(Wout[c0:c0 + CG, :].astype(np.float16)),
        "kb": np.ascontiguousarray(kbias.reshape(JTC, 128).T),
    }


def kernel(q, context, mask, Wq, Wkv, Wout, b_out):
    from concourse.bass_utils import run_bass_kernel_spmd

    q = np.asarray(q, dtype=np.float32)
    context = np.asarray(context, dtype=np.float32)
    mask = np.asarray(mask)
    Wq = np.asarray(Wq, dtype=np.float32)
    Wkv = np.asarray(Wkv, dtype=np.float32)
    Wout = np.asarray(Wout, dtype=np.float32)
    b_out = np.asarray(b_out, dtype=np.float32)

    nkc = _nkc_for_mask(mask)
    key = ("nc", nkc)
    if key not in _CACHE:
        _CACHE[key] = build_nc(nkc=nkc)
    nc = _CACHE[key]
    _CACHE["nc"] = nc
    _CACHE["nkc"] = nkc

    in_maps = [
        _prep_core_inputs(q, context, mask, Wq, Wkv, Wout, c, nkc=nkc)
        for c in range(N_CORES)
    ]

    trace = bool(int(os.environ.get("BASS_ATTN_TRACE", "0")))
    res = run_bass_kernel_spmd(nc, in_maps, list(range(N_CORES)), trace=trace)
    _CACHE["last_results"] = res
    _CACHE["last_in_maps"] = in_maps

    out = np.empty((B, NQ, D), dtype=np.float32)
    for bi in range(B):
        acc = res.results[4 * bi]["outp"].astype(np.float32).copy()
        for g in range(1, 4):
            acc += res.results[4 * bi + g]["outp"].astype(np.float32)
        out[bi] = acc + b_out[None, :]
    return out
